# revision 62
# baseline (speedup 1.0000x reference)
"""CoStGcnBlock Trainium2 kernel.

Math (eval-mode, BN folded on host):
    Aw_s   = A_s * attn_s
    y      = relu(x + sum_s wg'_s @ (x . Aw_s) + b1eff)     (graph conv + BN1 + residual)
    out    = relu(x + conv_T(y, wt') + b2eff)               (9-tap temporal conv + BN2 + residual)

Device pipeline per (sample, 256-frame pair = 2 chunks of TC frames + 4-frame halos):
    1. DMA x fp32 packed -> SBUF [128=(chunk,c), (t,v25)].
    2. GpSimd cast -> bf16 xb [128, (t,w32)] (pad lanes read past-frame junk,
       killed later by zero weight rows); DVE fills lane 31 with b1eff.
    3. Channel mix (bf16): per 512-col group, 2 row-tiled [64K,128M,512N]
       matmuls (chunk in rows 0-63/64-127) x 2 o-halves; id cols carry the
       x-residual, bias col row picks up b1eff lane.
    4. Drain u PSUM -> uT, split across engines: half the slots DVE
       stream-transposed straight to f32r SBUF; half ACT-copied to bf16 SBUF
       then DVE-transposed (bf16 transposes run in fast DVE mode).
    5. Graph mix: k=128 matmul with AwS rows (s,w)+id -> zT[v32,(t,o32)] PSUM,
       4 col-tiled slots; ACT relu -> yT bf16; DVE transpose back ->
       y[(chunk,o64), (t,v32)].
    6. Temporal conv (bf16): per 20-frame group, per chunk one [64K,64M,500N]
       matmul per tap, accumulated over 9 taps; fp32r identity matmul adds the
       x residual; ACT relu with per-partition bias b2eff -> fp32 out; DMA out.

Sharding: data-parallel over batch N: core k processes samples 4k..4k+3.
"""

import numpy as np
import ml_dtypes

N, C, O, T, V, K = 32, 64, 64, 1024, 25, 9
NCORES = 8
NS = N // NCORES
BN_EPS = 1e-5
HALO = 4
W32 = 32

BF16 = ml_dtypes.bfloat16


def _fold_weights(A, attn, wg, bg, bn1_gamma, bn1_beta, bn1_mean, bn1_var,
                  wt, bt, bn2_gamma, bn2_beta, bn2_mean, bn2_var):
    """Host-side constant folding. Returns device weight arrays."""
    f32 = np.float32
    Aw = (A * attn).astype(f32)                                   # (3, V, V)
    inv1 = (bn1_gamma / np.sqrt(bn1_var + BN_EPS)).astype(f32)    # (O,)
    b1eff = (bg.sum(0) * inv1 + bn1_beta - bn1_mean * inv1).astype(f32)
    wgp = (wg * inv1[None, :, None]).astype(f32)                  # (3, O, C)
    inv2 = (bn2_gamma / np.sqrt(bn2_var + BN_EPS)).astype(f32)
    wtp = (wt[:, :, :, 0] * inv2[:, None, None]).astype(f32)      # (O, C, K)
    b2eff = (bt * inv2 + bn2_beta - bn2_mean * inv2).astype(f32)

    # wch [128, 256]: rows = c (duplicated 64..127); col 128*h + 32*b + j:
    #   b<3 -> wgp[b, 32h+j, c];  b=3 (identity) -> 1 if c == 32h+j.
    wch = np.zeros((64, 256), f32)
    for h in range(2):
        for b in range(3):
            wch[:, 128 * h + 32 * b:128 * h + 32 * b + 32] = wgp[b, 32 * h:32 * h + 32, :].T
        for j in range(32):
            wch[32 * h + j, 128 * h + 96 + j] = 1.0
    wch = np.concatenate([wch, wch], axis=0)

    # aws [128, 32]: rows 32b + w: b<3 -> Aw[b][w, v] (w<25); id block: delta_wv, row 31 = 1.
    aws = np.zeros((128, 32), f32)
    for b in range(3):
        aws[32 * b:32 * b + 25, :25] = Aw[b]
    for w in range(25):
        aws[96 + w, w] = 1.0
    aws[96 + 31, :25] = 1.0                                       # bias row
    # wt9 [128, 9*64]: rows = c (dup); col 64k + o = wtp[o, c, k]
    wt9 = np.zeros((64, 9 * 64), f32)
    for k in range(K):
        wt9[:, 64 * k:64 * k + 64] = wtp[:, :, k].T
    wt9 = np.concatenate([wt9, wt9], axis=0)

    # wt2 [128, 4*64]: tap-pair weights: row r = (tap parity r//64, c=r%64);
    # col 64*kp + o = wtp[o, c, 2*kp + r//64]
    wt2 = np.zeros((128, 4 * 64), f32)
    for kp in range(4):
        wt2[:64, 64 * kp:64 * kp + 64] = wtp[:, :, 2 * kp].T
        wt2[64:, 64 * kp:64 * kp + 64] = wtp[:, :, 2 * kp + 1].T

    b1t = np.concatenate([b1eff, b1eff]).reshape(128, 1)
    b2t = np.concatenate([b2eff, b2eff]).reshape(128, 1).astype(f32)

    return {
        "wch": wch.astype(BF16),
        "aws": aws.astype(f32),
        "awsb": aws.astype(BF16),
        "wt9": wt9.astype(BF16),
        "wt2": wt2.astype(BF16),
        "i128": np.eye(128, dtype=BF16),
        "z128": np.zeros((128, 128), f32),
        "b1t": b1t.astype(BF16),
        "b2t": b2t,
    }


def _apv(tile_ap, p0, pn, free_off, free_dims):
    """Strided view of an SBUF/PSUM tile: partitions [p0, p0+pn), given free dims."""
    import concourse.bass as bass
    pitch = tile_ap.ap[0][0]
    return bass.AP(tensor=tile_ap.tensor, offset=p0 * pitch + free_off,
                   ap=[[pitch, pn]] + [list(d) for d in free_dims])


def _build_program(ns=NS, t_total=T, tc=128):
    """Build the Bass program for one core processing `ns` samples of `t_total` frames."""
    import concourse.bass as bass
    import concourse.mybir as mybir
    import concourse.tile as tile
    from concourse import bacc

    dt = mybir.dt
    AF = mybir.ActivationFunctionType

    th = tc + 2 * HALO            # frames per chunk incl halo
    f_x = th * V                  # x free size per chunk (packed v)
    f_u = th * W32                # u/uT/yT/y free size (padded v/w stride 32)
    f_out = tc * V
    npairs = t_total // (2 * tc)
    n_groups = (f_u + 511) // 512           # 512-col groups over f_u
    CT = 20                                  # conv out frames per chunk
    n_cchunks = (tc + CT - 1) // CT

    nc = bacc.Bacc("TRN2", target_bir_lowering=False, debug=False, num_devices=NCORES)

    xs = nc.dram_tensor("xs", [ns, C, t_total, V], dt.float32, kind="ExternalInput")
    wch_d = nc.dram_tensor("wch", [128, 256], dt.bfloat16, kind="ExternalInput")
    aws_d = nc.dram_tensor("aws", [128, 32], dt.float32, kind="ExternalInput")
    awsb_d = nc.dram_tensor("awsb", [128, 32], dt.bfloat16, kind="ExternalInput")
    wt9_d = nc.dram_tensor("wt9", [128, 9 * 64], dt.bfloat16, kind="ExternalInput")
    wt2_d = nc.dram_tensor("wt2", [128, 4 * 64], dt.bfloat16, kind="ExternalInput")
    i128_d = nc.dram_tensor("i128", [128, 128], dt.bfloat16, kind="ExternalInput")
    b1t_d = nc.dram_tensor("b1t", [128, 1], dt.bfloat16, kind="ExternalInput")
    b2t_d = nc.dram_tensor("b2t", [128, 1], dt.float32, kind="ExternalInput")
    z128_d = nc.dram_tensor("z128", [128, 128], dt.float32, kind="ExternalInput")
    out_d = nc.dram_tensor("out", [ns, C, t_total, V], dt.float32, kind="ExternalOutput")

    ctv = C * t_total * V
    tv = t_total * V
    f32r = dt.float32r

    # channel-MM emission order alternates ch so consecutive MMs use different
    # PE row halves (concurrent streams); the graph-mix col position j must
    # stay ch-major (j determines the y partition layout (ch, o64)).
    slots_mm = [(0, 0), (1, 0), (0, 1), (1, 1)]
    slots = [(0, 0), (0, 1), (1, 0), (1, 1)]

    with tile.TileContext(nc) as tcx:
        import contextlib
        with contextlib.ExitStack() as ctx:
            const = ctx.enter_context(tcx.tile_pool(name="const", bufs=1))
            px = ctx.enter_context(tcx.tile_pool(name="px", bufs=2))
            pxb = ctx.enter_context(tcx.tile_pool(name="pxb", bufs=3))
            put = ctx.enter_context(tcx.tile_pool(name="put", bufs=8))
            pyt = ctx.enter_context(tcx.tile_pool(name="pyt", bufs=2))
            py = ctx.enter_context(tcx.tile_pool(name="py", bufs=2))
            py2 = ctx.enter_context(tcx.tile_pool(name="py2", bufs=2))
            pout = ctx.enter_context(tcx.tile_pool(name="pout", bufs=2))
            ppu = ctx.enter_context(tcx.tile_pool(name="ppu", bufs=4, space="PSUM"))
            ppg = ctx.enter_context(tcx.tile_pool(name="ppg", bufs=2, space="PSUM"))
            ppc = ctx.enter_context(tcx.tile_pool(name="ppc", bufs=2, space="PSUM"))

            c_wch = const.tile([128, 256], dt.bfloat16)
            nc.sync.dma_start(out=c_wch[:], in_=wch_d[:])
            c_aws = const.tile([128, 32], dt.float32)
            nc.sync.dma_start(out=c_aws[:], in_=aws_d[:])
            c_awsb = const.tile([128, 32], dt.bfloat16)
            nc.sync.dma_start(out=c_awsb[:], in_=awsb_d[:])
            c_wt9 = const.tile([128, 9 * 64], dt.bfloat16)
            nc.sync.dma_start(out=c_wt9[:], in_=wt9_d[:])
            c_wt2 = const.tile([128, 4 * 64], dt.bfloat16)
            nc.sync.dma_start(out=c_wt2[:], in_=wt2_d[:])
            c_i128 = const.tile([128, 128], dt.bfloat16)
            nc.sync.dma_start(out=c_i128[:], in_=i128_d[:])
            c_b1t = const.tile([128, 1], dt.bfloat16)
            nc.sync.dma_start(out=c_b1t[:], in_=b1t_d[:])
            c_b2t = const.tile([128, 1], dt.float32)
            nc.sync.dma_start(out=c_b2t[:], in_=b2t_d[:])

            slot_of = {key: j for j, key in enumerate(slots)}
            pend = None   # conv state of the previous pair (software pipeline)

            def emit_conv_g3(st):
                """Emit one 20-frame conv chunk of the previous pair; these are
                interleaved into the next pair's drain-paced channel phase so
                the PE stays dense (keeps HAM at full clock)."""
                g3 = st["g3"]
                st["g3"] += 1
                nt = min(CT, tc - g3 * CT)
                cols = nt * V
                c_ps = ppc.tile([128, 500], dt.float32, tag="cpsum",
                                name=f"cps{st['n']}_{st['p']}_{g3}")
                for kp in range(4):
                    for ch in range(2):
                        rhs = _apv(st["y2"][ch][:], 0, 128,
                                   (g3 * CT + 2 * kp) * W32,
                                   [[W32, nt], [1, V]])
                        for oh in range(2):
                            jj = 2 * ch + oh
                            nc.tensor.matmul(
                                c_ps[32 * jj:32 * jj + 32, 0:cols],
                                c_wt2[:, 64 * kp + 32 * oh:
                                      64 * kp + 32 * oh + 32],
                                rhs, start=(kp == 0), stop=False,
                                tile_position=(0, 32 * jj))
                # tap 8 (single, 64-contraction straight from y)
                for ch in range(2):
                    rhs = _apv(st["y"][:], 64 * ch, 64, (g3 * CT + 8) * W32,
                               [[W32, nt], [1, V]])
                    for oh in range(2):
                        jj = 2 * ch + oh
                        nc.tensor.matmul(
                            c_ps[32 * jj:32 * jj + 32, 0:cols],
                            c_wt9[64 * ch:64 * ch + 64,
                                  64 * 8 + 32 * oh:64 * 8 + 32 * oh + 32],
                            rhs, start=False, stop=False,
                            tile_position=(64 * ch, 32 * jj),
                            skip_group_check=True)
                x_res = _apv(st["xb"][:], 0, 128, (HALO + g3 * CT) * W32,
                             [[W32, nt], [1, V]])
                nc.tensor.matmul(c_ps[:, 0:cols], c_i128[:], x_res,
                                 start=False, stop=True, tile_position=(0, 0),
                                 skip_group_check=True)
                nc.scalar.activation(
                    out=st["out_sb"][:, g3 * CT * V:g3 * CT * V + cols],
                    in_=c_ps[:, 0:cols], func=AF.Relu,
                    bias=c_b2t[:, 0:1], scale=1.0)

            def emit_conv_out(st):
                for ch in range(2):
                    out_ap = bass.AP(
                        tensor=out_d,
                        offset=st["n"] * ctv + (st["t0"] + tc * ch) * V,
                        ap=[[tv, C], [1, f_out]])
                    nc.sync.dma_start(
                        out=out_ap,
                        in_=_apv(st["out_sb"][:], 64 * ch, 64, 0, [[1, f_out]]))

            def emit_front(n, p):
                    """DMA x + padded bf16 cast + bias lane for pair (n, p)."""
                    t0 = p * 2 * tc
                    x_sb = px.tile([128, f_x + 8], f32r, tag="x",
                                   name=f"x{n}_{p}")

                    first = p == 0
                    last = p == npairs - 1
                    if not first and not last:
                        # per-chunk DMAs: outer dim C=64 spreads packets over all
                        # SDMA engines (a fused [2,C,...] AP lands on only 2).
                        for ch in range(2):
                            in_ap = bass.AP(
                                tensor=xs,
                                offset=n * ctv + (t0 + tc * ch - HALO) * V,
                                ap=[[tv, C], [1, f_x + 8]])
                            nc.sync.dma_start(
                                out=_apv(x_sb[:], 64 * ch, 64, 0, [[1, f_x + 8]]),
                                in_=in_ap.bitcast(f32r))
                    else:
                        for ch in range(2):
                            tlo = t0 + tc * ch - HALO
                            thi = tlo + th
                            vlo, vhi = max(tlo, 0), min(thi + 1, t_total)
                            doff = (vlo - tlo) * V
                            dlen = (vhi - vlo) * V
                            dlen = min(dlen, f_x + 8 - doff)
                            if doff > 0:   # zero-fill left gap
                                nc.sync.dma_start(
                                    out=_apv(x_sb[:], 64 * ch, 64, 0, [[1, doff]]),
                                    in_=_apv(z128_d.ap(), 0, 64, 0,
                                             [[0, (doff + 127) // 128],
                                              [1, min(doff, 128)]]).bitcast(f32r)
                                    if doff > 128 else
                                    _apv(z128_d.ap(), 0, 64, 0,
                                         [[1, doff]]).bitcast(f32r))
                            if doff + dlen < f_x + 8:   # zero-fill right gap + tail
                                glen = f_x + 8 - (doff + dlen)
                                nc.sync.dma_start(
                                    out=_apv(x_sb[:], 64 * ch, 64, doff + dlen, [[1, glen]]),
                                    in_=_apv(z128_d.ap(), 0, 64, 0,
                                             [[0, (glen + 127) // 128],
                                              [1, min(glen, 128)]]).bitcast(f32r))
                            in_ap = bass.AP(tensor=xs, offset=n * ctv + vlo * V,
                                            ap=[[tv, C], [1, dlen]])
                            out_ap = _apv(x_sb[:], 64 * ch, 64, doff, [[1, dlen]])
                            nc.sync.dma_start(out=out_ap, in_=in_ap.bitcast(f32r))

                    # bf16 (t,w32)-padded copy of x: each frame reads 32
                    # consecutive packed elems (7 spill into the next frame;
                    # junk is killed by zero aws rows). Lane 31 then carries
                    # b1eff via the id matmul.
                    xb = pxb.tile([128, f_u], dt.bfloat16, tag="xb",
                                  name=f"xb{n}_{p}")
                    # cast on the otherwise-idle GpSimd: slower there (~15us)
                    # but prefetched a pair ahead, and it keeps the ACT queue
                    # free for the relu1 chain (which gates the back-transpose)
                    nc.gpsimd.tensor_copy(
                        out=_apv(xb[:], 0, 128, 0, [[1, f_u]]),
                        in_=_apv(x_sb[:], 0, 128, 0,
                                 [[V, th], [1, W32]]).bitcast(dt.float32))
                    nc.scalar.activation(
                        out=_apv(xb[:], 0, 128, 31, [[W32, th]]),
                        in_=_apv(c_b1t[:], 0, 128, 0, [[0, th]]).bitcast(dt.bfloat16),
                        func=AF.Copy, bias=0.0, scale=1.0)
                    return xb

            pairs = [(n, p) for n in range(ns) for p in range(npairs)]
            fronts = {}
            for idx, (n, p) in enumerate(pairs):
                    t0 = p * 2 * tc
                    first = p == 0
                    last = p == npairs - 1
                    if (n, p) not in fronts:
                        fronts[(n, p)] = emit_front(n, p)
                    xb = fronts.pop((n, p))

                    yt = pyt.tile([128, f_u], dt.bfloat16, tag="yt")

                    # ---- channel mix + drain + graph mix + relu, per group ----
                    for gp in range(n_groups):
                        cols = min(512, f_u - gp * 512)
                        uts = {}
                        for ch, h in slots_mm:
                            u_ps = ppu.tile([128, 512], dt.float32, tag="upsum")
                            rhs = _apv(xb[:], 64 * ch, 64, gp * 512, [[1, cols]])
                            nc.tensor.matmul(
                                u_ps[:, 0:cols],
                                c_wch[64 * ch:64 * ch + 64,
                                      128 * h:128 * h + 128],
                                rhs, start=True, stop=True,
                                tile_position=(64 * ch, 0))
                            ut = put.tile([128, 512], dt.float32, tag="ut")
                            nc.vector.transpose(out=ut[:, 0:cols],
                                                in_=u_ps[:, 0:cols])
                            uts[(ch, h)] = ut

                        # graph mix (+ residual + bias via id rows); the fp32
                        # uT is read as bf16 via its high 2 bytes (stride-2
                        # bitcast view) so the matmul runs at 1 cycle/row.
                        g_ps = ppg.tile([128, 512], dt.float32, tag="gpsum")
                        for j, key in enumerate(slots):
                            bap = uts[key][:].bitcast(dt.bfloat16)
                            hi = bass.AP(tensor=bap.tensor,
                                         offset=bap.offset + 1,
                                         ap=[[bap.ap[0][0], 128], [2, cols]])
                            nc.tensor.matmul(
                                g_ps[32 * j:32 * j + 32, 0:cols],
                                c_awsb[:, 0:32], hi,
                                start=True, stop=True, tile_position=(0, 32 * j))
                        nc.scalar.activation(out=yt[:, gp * 512:gp * 512 + cols],
                                             in_=g_ps[:, 0:cols], func=AF.Relu,
                                             bias=0.0, scale=1.0)
                        # interleave one conv chunk of the previous pair
                        if pend is not None and pend["g3"] < n_cchunks and gp >= 1:
                            emit_conv_g3(pend)
                        # prefetch the next pair's input mid-loop so its cast
                        # doesn't serialize the pair boundary on ACT
                        if gp == 4 and idx + 1 < len(pairs):
                            fronts[pairs[idx + 1]] = emit_front(*pairs[idx + 1])

                    if pend is not None:
                        while pend["g3"] < n_cchunks:
                            emit_conv_g3(pend)
                        emit_conv_out(pend)

                    # transpose back -> y [(ch, o64), (t, v32)]
                    y = py.tile([128, f_u], dt.bfloat16, tag="y")
                    nc.vector.transpose(out=y[:], in_=yt[:])
                    if first:
                        nc.vector.memset(y[0:64, 0:HALO * W32], 0.0)
                    if last:
                        nc.vector.memset(y[64:128, (th - HALO) * W32:f_u], 0.0)

                    # y2[ch]: rows 0-63 = y chunk ch, rows 64-127 = same
                    # shifted one frame (tap-pair packing for 128-contraction)
                    y2 = []
                    for ch in range(2):
                        t2 = py2.tile([128, f_u], dt.bfloat16, tag=f"y2{ch}")
                        nc.sync.dma_start(
                            out=_apv(t2[:], 0, 64, 0, [[1, f_u]]),
                            in_=_apv(y[:], 64 * ch, 64, 0, [[1, f_u]]))
                        nc.sync.dma_start(
                            out=_apv(t2[:], 64, 64, 0, [[1, f_u - W32]]),
                            in_=_apv(y[:], 64 * ch, 64, W32, [[1, f_u - W32]]))
                        y2.append(t2)

                    out_sb = pout.tile([128, f_out], dt.float32, tag="o")
                    pend = {"n": n, "p": p, "t0": t0, "xb": xb, "y": y,
                            "y2": y2, "out_sb": out_sb, "g3": 0}

            # pipeline epilogue: conv of the final pair
            while pend["g3"] < n_cchunks:
                emit_conv_g3(pend)
            emit_conv_out(pend)

    nc.finalize()
    return nc


_CACHE = {}


def _get_program(ns, t_total, tc, **kw):
    key = (ns, t_total, tc, tuple(sorted(kw.items())))
    if key not in _CACHE:
        _CACHE[key] = _build_program(ns, t_total, tc, **kw)
    return _CACHE[key]


def kernel(**inputs):
    from concourse.bass_utils import run_bass_kernel_spmd

    x = np.asarray(inputs["x"], dtype=np.float32)
    w = _fold_weights(
        np.asarray(inputs["A"]), np.asarray(inputs["attn"]),
        np.asarray(inputs["wg"]), np.asarray(inputs["bg"]),
        np.asarray(inputs["bn1_gamma"]), np.asarray(inputs["bn1_beta"]),
        np.asarray(inputs["bn1_mean"]), np.asarray(inputs["bn1_var"]),
        np.asarray(inputs["wt"]), np.asarray(inputs["bt"]),
        np.asarray(inputs["bn2_gamma"]), np.asarray(inputs["bn2_beta"]),
        np.asarray(inputs["bn2_mean"]), np.asarray(inputs["bn2_var"]))

    nc = _get_program(NS, T, 128)
    in_maps = []
    for k in range(NCORES):
        m = {"xs": np.ascontiguousarray(x[NS * k:NS * k + NS])}
        m.update(w)
        in_maps.append(m)
    res = run_bass_kernel_spmd(nc, in_maps, core_ids=list(range(NCORES)))
    return np.concatenate([r["out"] for r in res.results], axis=0)


# revision 63
# speedup vs baseline: 1.3248x; 1.3248x over previous
"""CoStGcnBlock Trainium2 kernel.

Math (eval-mode, BN folded on host):
    Aw_s   = A_s * attn_s
    y      = relu(x + sum_s wg'_s @ (x . Aw_s) + b1eff)     (graph conv + BN1 + residual)
    out    = relu(x + conv_T(y, wt') + b2eff)               (9-tap temporal conv + BN2 + residual)

Device pipeline per (sample, 256-frame pair = 2 chunks of TC frames + 4-frame halos):
    1. DMA x fp32 packed -> SBUF [128=(chunk,c), (t,v25)].
    2. GpSimd cast -> bf16 xb [128, (t,w32)] (pad lanes read past-frame junk,
       killed later by zero weight rows); DVE fills lane 31 with b1eff.
    3. Channel mix (bf16): per 512-col group, 2 row-tiled [64K,128M,512N]
       matmuls (chunk in rows 0-63/64-127) x 2 o-halves; id cols carry the
       x-residual, bias col row picks up b1eff lane.
    4. Drain u PSUM -> uT, split across engines: half the slots DVE
       stream-transposed straight to f32r SBUF; half ACT-copied to bf16 SBUF
       then DVE-transposed (bf16 transposes run in fast DVE mode).
    5. Graph mix: k=128 matmul with AwS rows (s,w)+id -> zT[v32,(t,o32)] PSUM,
       4 col-tiled slots; ACT relu -> yT bf16; DVE transpose back ->
       y[(chunk,o64), (t,v32)].
    6. Temporal conv (bf16): per 20-frame group, per chunk one [64K,64M,500N]
       matmul per tap, accumulated over 9 taps; fp32r identity matmul adds the
       x residual; ACT relu with per-partition bias b2eff -> fp32 out; DMA out.

Sharding: data-parallel over batch N: core k processes samples 4k..4k+3.
"""

import numpy as np
import ml_dtypes

N, C, O, T, V, K = 32, 64, 64, 1024, 25, 9
NCORES = 8
NS = N // NCORES
BN_EPS = 1e-5
HALO = 4
W32 = 32

BF16 = ml_dtypes.bfloat16


def _fold_weights(A, attn, wg, bg, bn1_gamma, bn1_beta, bn1_mean, bn1_var,
                  wt, bt, bn2_gamma, bn2_beta, bn2_mean, bn2_var):
    """Host-side constant folding. Returns device weight arrays."""
    f32 = np.float32
    Aw = (A * attn).astype(f32)                                   # (3, V, V)
    inv1 = (bn1_gamma / np.sqrt(bn1_var + BN_EPS)).astype(f32)    # (O,)
    b1eff = (bg.sum(0) * inv1 + bn1_beta - bn1_mean * inv1).astype(f32)
    wgp = (wg * inv1[None, :, None]).astype(f32)                  # (3, O, C)
    inv2 = (bn2_gamma / np.sqrt(bn2_var + BN_EPS)).astype(f32)
    wtp = (wt[:, :, :, 0] * inv2[:, None, None]).astype(f32)      # (O, C, K)
    b2eff = (bt * inv2 + bn2_beta - bn2_mean * inv2).astype(f32)

    # wch [128, 256]: rows = c (duplicated 64..127); col 128*h + 32*b + j:
    #   b<3 -> wgp[b, 32h+j, c];  b=3 (identity) -> 1 if c == 32h+j.
    wch = np.zeros((64, 256), f32)
    for h in range(2):
        for b in range(3):
            wch[:, 128 * h + 32 * b:128 * h + 32 * b + 32] = wgp[b, 32 * h:32 * h + 32, :].T
        for j in range(32):
            wch[32 * h + j, 128 * h + 96 + j] = 1.0
    wch = np.concatenate([wch, wch], axis=0)

    # aws [128, 32]: rows 32b + w: b<3 -> Aw[b][w, v] (w<25); id block: delta_wv, row 31 = 1.
    aws = np.zeros((128, 32), f32)
    for b in range(3):
        aws[32 * b:32 * b + 25, :25] = Aw[b]
    for w in range(25):
        aws[96 + w, w] = 1.0
    aws[96 + 31, :25] = 1.0                                       # bias row
    # wt9 [128, 9*64]: rows = c (dup); col 64k + o = wtp[o, c, k]
    wt9 = np.zeros((64, 9 * 64), f32)
    for k in range(K):
        wt9[:, 64 * k:64 * k + 64] = wtp[:, :, k].T
    wt9 = np.concatenate([wt9, wt9], axis=0)

    # wt2 [128, 4*64]: tap-pair weights: row r = (tap parity r//64, c=r%64);
    # col 64*kp + o = wtp[o, c, 2*kp + r//64]
    wt2 = np.zeros((128, 4 * 64), f32)
    for kp in range(4):
        wt2[:64, 64 * kp:64 * kp + 64] = wtp[:, :, 2 * kp].T
        wt2[64:, 64 * kp:64 * kp + 64] = wtp[:, :, 2 * kp + 1].T

    b1t = np.concatenate([b1eff, b1eff]).reshape(128, 1)
    b2t = np.concatenate([b2eff, b2eff]).reshape(128, 1).astype(f32)

    return {
        "wch": wch.astype(BF16),
        "aws": aws.astype(f32),
        "awsb": aws.astype(BF16),
        "wt9": wt9.astype(BF16),
        "wt2": wt2.astype(BF16),
        "i128": np.eye(128, dtype=BF16),
        "z128": np.zeros((128, 128), f32),
        "b1t": b1t.astype(BF16),
        "b2t": b2t,
    }


def _apv(tile_ap, p0, pn, free_off, free_dims):
    """Strided view of an SBUF/PSUM tile: partitions [p0, p0+pn), given free dims."""
    import concourse.bass as bass
    pitch = tile_ap.ap[0][0]
    return bass.AP(tensor=tile_ap.tensor, offset=p0 * pitch + free_off,
                   ap=[[pitch, pn]] + [list(d) for d in free_dims])


def _build_program(ns=NS, t_total=T, tc=128):
    """Build the Bass program for one core processing `ns` samples of `t_total` frames."""
    import concourse.bass as bass
    import concourse.mybir as mybir
    import concourse.tile as tile
    from concourse import bacc

    dt = mybir.dt
    AF = mybir.ActivationFunctionType

    th = tc + 2 * HALO            # frames per chunk incl halo
    f_x = th * V                  # x free size per chunk (packed v)
    f_u = th * W32                # u/uT/yT/y free size (padded v/w stride 32)
    f_out = tc * V
    npairs = t_total // (2 * tc)
    n_groups = (f_u + 511) // 512           # 512-col groups over f_u
    CT = 20                                  # conv out frames per chunk
    n_cchunks = (tc + CT - 1) // CT

    nc = bacc.Bacc("TRN2", target_bir_lowering=False, debug=False, num_devices=NCORES)

    xs = nc.dram_tensor("xs", [ns, C, t_total, V], dt.float32, kind="ExternalInput")
    wch_d = nc.dram_tensor("wch", [128, 256], dt.bfloat16, kind="ExternalInput")
    aws_d = nc.dram_tensor("aws", [128, 32], dt.float32, kind="ExternalInput")
    awsb_d = nc.dram_tensor("awsb", [128, 32], dt.bfloat16, kind="ExternalInput")
    wt9_d = nc.dram_tensor("wt9", [128, 9 * 64], dt.bfloat16, kind="ExternalInput")
    wt2_d = nc.dram_tensor("wt2", [128, 4 * 64], dt.bfloat16, kind="ExternalInput")
    i128_d = nc.dram_tensor("i128", [128, 128], dt.bfloat16, kind="ExternalInput")
    b1t_d = nc.dram_tensor("b1t", [128, 1], dt.bfloat16, kind="ExternalInput")
    b2t_d = nc.dram_tensor("b2t", [128, 1], dt.float32, kind="ExternalInput")
    z128_d = nc.dram_tensor("z128", [128, 128], dt.float32, kind="ExternalInput")
    out_d = nc.dram_tensor("out", [ns, C, t_total, V], dt.float32, kind="ExternalOutput")

    ctv = C * t_total * V
    tv = t_total * V
    f32r = dt.float32r

    # channel-MM emission order alternates ch so consecutive MMs use different
    # PE row halves (concurrent streams); the graph-mix col position j must
    # stay ch-major (j determines the y partition layout (ch, o64)).
    slots_mm = [(0, 0), (1, 0), (0, 1), (1, 1)]
    slots = [(0, 0), (0, 1), (1, 0), (1, 1)]

    with tile.TileContext(nc) as tcx:
        import contextlib
        with contextlib.ExitStack() as ctx:
            const = ctx.enter_context(tcx.tile_pool(name="const", bufs=1))
            px = ctx.enter_context(tcx.tile_pool(name="px", bufs=2))
            pxb = ctx.enter_context(tcx.tile_pool(name="pxb", bufs=3))
            put = ctx.enter_context(tcx.tile_pool(name="put", bufs=8))
            pyt = ctx.enter_context(tcx.tile_pool(name="pyt", bufs=2))
            py = ctx.enter_context(tcx.tile_pool(name="py", bufs=2))
            py2 = ctx.enter_context(tcx.tile_pool(name="py2", bufs=2))
            pout = ctx.enter_context(tcx.tile_pool(name="pout", bufs=2))
            ppu = ctx.enter_context(tcx.tile_pool(name="ppu", bufs=4, space="PSUM"))
            ppg = ctx.enter_context(tcx.tile_pool(name="ppg", bufs=2, space="PSUM"))
            ppc = ctx.enter_context(tcx.tile_pool(name="ppc", bufs=2, space="PSUM"))

            c_wch = const.tile([128, 256], dt.bfloat16)
            nc.sync.dma_start(out=c_wch[:], in_=wch_d[:])
            c_aws = const.tile([128, 32], dt.float32)
            nc.sync.dma_start(out=c_aws[:], in_=aws_d[:])
            c_awsb = const.tile([128, 32], dt.bfloat16)
            nc.sync.dma_start(out=c_awsb[:], in_=awsb_d[:])
            c_wt9 = const.tile([128, 9 * 64], dt.bfloat16)
            nc.sync.dma_start(out=c_wt9[:], in_=wt9_d[:])
            c_wt2 = const.tile([128, 4 * 64], dt.bfloat16)
            nc.sync.dma_start(out=c_wt2[:], in_=wt2_d[:])
            c_i128 = const.tile([128, 128], dt.bfloat16)
            nc.sync.dma_start(out=c_i128[:], in_=i128_d[:])
            c_b1t = const.tile([128, 1], dt.bfloat16)
            nc.sync.dma_start(out=c_b1t[:], in_=b1t_d[:])
            c_b2t = const.tile([128, 1], dt.float32)
            nc.sync.dma_start(out=c_b2t[:], in_=b2t_d[:])

            slot_of = {key: j for j, key in enumerate(slots)}
            pend = None   # conv state of the previous pair (software pipeline)

            def emit_conv_g3(st):
                """Emit one 20-frame conv chunk of the previous pair; these are
                interleaved into the next pair's drain-paced channel phase so
                the PE stays dense (keeps HAM at full clock)."""
                g3 = st["g3"]
                st["g3"] += 1
                nt = min(CT, tc - g3 * CT)
                cols = nt * V
                c_ps = ppc.tile([128, 500], dt.float32, tag="cpsum",
                                name=f"cps{st['n']}_{st['p']}_{g3}")
                for kp in range(4):
                    for ch in range(2):
                        rhs = _apv(st["y2"][ch][:], 0, 128,
                                   (g3 * CT + 2 * kp) * W32,
                                   [[W32, nt], [1, V]])
                        for oh in range(2):
                            jj = 2 * ch + oh
                            nc.tensor.matmul(
                                c_ps[32 * jj:32 * jj + 32, 0:cols],
                                c_wt2[:, 64 * kp + 32 * oh:
                                      64 * kp + 32 * oh + 32],
                                rhs, start=(kp == 0), stop=False,
                                tile_position=(0, 32 * jj))
                # tap 8 (single, 64-contraction straight from y)
                for ch in range(2):
                    rhs = _apv(st["y"][:], 64 * ch, 64, (g3 * CT + 8) * W32,
                               [[W32, nt], [1, V]])
                    for oh in range(2):
                        jj = 2 * ch + oh
                        nc.tensor.matmul(
                            c_ps[32 * jj:32 * jj + 32, 0:cols],
                            c_wt9[64 * ch:64 * ch + 64,
                                  64 * 8 + 32 * oh:64 * 8 + 32 * oh + 32],
                            rhs, start=False, stop=False,
                            tile_position=(64 * ch, 32 * jj),
                            skip_group_check=True)
                x_res = _apv(st["xb"][:], 0, 128, (HALO + g3 * CT) * W32,
                             [[W32, nt], [1, V]])
                nc.tensor.matmul(c_ps[:, 0:cols], c_i128[:], x_res,
                                 start=False, stop=True, tile_position=(0, 0),
                                 skip_group_check=True)
                nc.scalar.activation(
                    out=st["out_sb"][:, g3 * CT * V:g3 * CT * V + cols],
                    in_=c_ps[:, 0:cols], func=AF.Relu,
                    bias=c_b2t[:, 0:1], scale=1.0)

            def emit_conv_out(st):
                for ch in range(2):
                    out_ap = bass.AP(
                        tensor=out_d,
                        offset=st["n"] * ctv + (st["t0"] + tc * ch) * V,
                        ap=[[tv, C], [1, f_out]])
                    nc.sync.dma_start(
                        out=out_ap,
                        in_=_apv(st["out_sb"][:], 64 * ch, 64, 0, [[1, f_out]]))

            def emit_front(n, p):
                    """DMA x + padded bf16 cast + bias lane for pair (n, p)."""
                    t0 = p * 2 * tc
                    x_sb = px.tile([128, f_x + 8], f32r, tag="x",
                                   name=f"x{n}_{p}")

                    first = p == 0
                    last = p == npairs - 1
                    if not first and not last:
                        # per-chunk DMAs: outer dim C=64 spreads packets over all
                        # SDMA engines (a fused [2,C,...] AP lands on only 2).
                        for ch in range(2):
                            in_ap = bass.AP(
                                tensor=xs,
                                offset=n * ctv + (t0 + tc * ch - HALO) * V,
                                ap=[[tv, C], [1, f_x + 8]])
                            nc.sync.dma_start(
                                out=_apv(x_sb[:], 64 * ch, 64, 0, [[1, f_x + 8]]),
                                in_=in_ap.bitcast(f32r))
                    else:
                        for ch in range(2):
                            tlo = t0 + tc * ch - HALO
                            thi = tlo + th
                            vlo, vhi = max(tlo, 0), min(thi + 1, t_total)
                            doff = (vlo - tlo) * V
                            dlen = (vhi - vlo) * V
                            dlen = min(dlen, f_x + 8 - doff)
                            if doff > 0:   # zero-fill left gap
                                nc.sync.dma_start(
                                    out=_apv(x_sb[:], 64 * ch, 64, 0, [[1, doff]]),
                                    in_=_apv(z128_d.ap(), 0, 64, 0,
                                             [[0, (doff + 127) // 128],
                                              [1, min(doff, 128)]]).bitcast(f32r)
                                    if doff > 128 else
                                    _apv(z128_d.ap(), 0, 64, 0,
                                         [[1, doff]]).bitcast(f32r))
                            if doff + dlen < f_x + 8:   # zero-fill right gap + tail
                                glen = f_x + 8 - (doff + dlen)
                                nc.sync.dma_start(
                                    out=_apv(x_sb[:], 64 * ch, 64, doff + dlen, [[1, glen]]),
                                    in_=_apv(z128_d.ap(), 0, 64, 0,
                                             [[0, (glen + 127) // 128],
                                              [1, min(glen, 128)]]).bitcast(f32r))
                            in_ap = bass.AP(tensor=xs, offset=n * ctv + vlo * V,
                                            ap=[[tv, C], [1, dlen]])
                            out_ap = _apv(x_sb[:], 64 * ch, 64, doff, [[1, dlen]])
                            nc.sync.dma_start(out=out_ap, in_=in_ap.bitcast(f32r))

                    # bf16 (t,w32)-padded copy of x: each frame reads 32
                    # consecutive packed elems (7 spill into the next frame;
                    # junk is killed by zero aws rows). Lane 31 then carries
                    # b1eff via the id matmul.
                    xb = pxb.tile([128, f_u], dt.bfloat16, tag="xb",
                                  name=f"xb{n}_{p}")
                    nc.scalar.activation(
                        out=xb[:, 0:f_u],
                        in_=_apv(x_sb[:], 0, 128, 0,
                                 [[V, th], [1, W32]]).bitcast(dt.float32),
                        func=AF.Copy, bias=0.0, scale=1.0)
                    nc.scalar.activation(
                        out=_apv(xb[:], 0, 128, 31, [[W32, th]]),
                        in_=_apv(c_b1t[:], 0, 128, 0, [[0, th]]).bitcast(dt.bfloat16),
                        func=AF.Copy, bias=0.0, scale=1.0)
                    return xb

            pairs = [(n, p) for n in range(ns) for p in range(npairs)]
            fronts = {}
            for idx, (n, p) in enumerate(pairs):
                    t0 = p * 2 * tc
                    first = p == 0
                    last = p == npairs - 1
                    if (n, p) not in fronts:
                        fronts[(n, p)] = emit_front(n, p)
                    xb = fronts.pop((n, p))

                    yt = pyt.tile([128, f_u], dt.bfloat16, tag="yt")

                    # ---- channel mix + drain + graph mix + relu, per group ----
                    for gp in range(n_groups):
                        cols = min(512, f_u - gp * 512)
                        uts = {}
                        for ch, h in slots_mm:
                            u_ps = ppu.tile([128, 512], dt.float32, tag="upsum")
                            rhs = _apv(xb[:], 64 * ch, 64, gp * 512, [[1, cols]])
                            nc.tensor.matmul(
                                u_ps[:, 0:cols],
                                c_wch[64 * ch:64 * ch + 64,
                                      128 * h:128 * h + 128],
                                rhs, start=True, stop=True,
                                tile_position=(64 * ch, 0))
                            ut = put.tile([128, 512], dt.float32, tag="ut")
                            nc.vector.transpose(out=ut[:, 0:cols],
                                                in_=u_ps[:, 0:cols])
                            uts[(ch, h)] = ut

                        # graph mix (+ residual + bias via id rows); the fp32
                        # uT is read as bf16 via its high 2 bytes (stride-2
                        # bitcast view) so the matmul runs at 1 cycle/row.
                        g_ps = ppg.tile([128, 512], dt.float32, tag="gpsum")
                        for j, key in enumerate(slots):
                            bap = uts[key][:].bitcast(dt.bfloat16)
                            hi = bass.AP(tensor=bap.tensor,
                                         offset=bap.offset + 1,
                                         ap=[[bap.ap[0][0], 128], [2, cols]])
                            nc.tensor.matmul(
                                g_ps[32 * j:32 * j + 32, 0:cols],
                                c_awsb[:, 0:32], hi,
                                start=True, stop=True, tile_position=(0, 32 * j))
                        nc.scalar.activation(out=yt[:, gp * 512:gp * 512 + cols],
                                             in_=g_ps[:, 0:cols], func=AF.Relu,
                                             bias=0.0, scale=1.0)
                        # interleave one conv chunk of the previous pair
                        if pend is not None and pend["g3"] < n_cchunks and gp >= 1:
                            emit_conv_g3(pend)
                        # prefetch the next pair's input mid-loop so its cast
                        # doesn't serialize the pair boundary on ACT
                        if gp == 4 and idx + 1 < len(pairs):
                            fronts[pairs[idx + 1]] = emit_front(*pairs[idx + 1])

                    if pend is not None:
                        while pend["g3"] < n_cchunks:
                            emit_conv_g3(pend)
                        emit_conv_out(pend)

                    # transpose back -> y [(ch, o64), (t, v32)]
                    y = py.tile([128, f_u], dt.bfloat16, tag="y")
                    nc.vector.transpose(out=y[:], in_=yt[:])
                    if first:
                        nc.vector.memset(y[0:64, 0:HALO * W32], 0.0)
                    if last:
                        nc.vector.memset(y[64:128, (th - HALO) * W32:f_u], 0.0)

                    # y2[ch]: rows 0-63 = y chunk ch, rows 64-127 = same
                    # shifted one frame (tap-pair packing for 128-contraction)
                    y2 = []
                    for ch in range(2):
                        t2 = py2.tile([128, f_u], dt.bfloat16, tag=f"y2{ch}")
                        nc.sync.dma_start(
                            out=_apv(t2[:], 0, 64, 0, [[1, f_u]]),
                            in_=_apv(y[:], 64 * ch, 64, 0, [[1, f_u]]))
                        nc.sync.dma_start(
                            out=_apv(t2[:], 64, 64, 0, [[1, f_u - W32]]),
                            in_=_apv(y[:], 64 * ch, 64, W32, [[1, f_u - W32]]))
                        y2.append(t2)

                    out_sb = pout.tile([128, f_out], dt.float32, tag="o")
                    pend = {"n": n, "p": p, "t0": t0, "xb": xb, "y": y,
                            "y2": y2, "out_sb": out_sb, "g3": 0}

            # pipeline epilogue: conv of the final pair
            while pend["g3"] < n_cchunks:
                emit_conv_g3(pend)
            emit_conv_out(pend)

    nc.finalize()
    return nc


_CACHE = {}


def _get_program(ns, t_total, tc, **kw):
    key = (ns, t_total, tc, tuple(sorted(kw.items())))
    if key not in _CACHE:
        _CACHE[key] = _build_program(ns, t_total, tc, **kw)
    return _CACHE[key]


def kernel(**inputs):
    from concourse.bass_utils import run_bass_kernel_spmd

    x = np.asarray(inputs["x"], dtype=np.float32)
    w = _fold_weights(
        np.asarray(inputs["A"]), np.asarray(inputs["attn"]),
        np.asarray(inputs["wg"]), np.asarray(inputs["bg"]),
        np.asarray(inputs["bn1_gamma"]), np.asarray(inputs["bn1_beta"]),
        np.asarray(inputs["bn1_mean"]), np.asarray(inputs["bn1_var"]),
        np.asarray(inputs["wt"]), np.asarray(inputs["bt"]),
        np.asarray(inputs["bn2_gamma"]), np.asarray(inputs["bn2_beta"]),
        np.asarray(inputs["bn2_mean"]), np.asarray(inputs["bn2_var"]))

    nc = _get_program(NS, T, 128)
    in_maps = []
    for k in range(NCORES):
        m = {"xs": np.ascontiguousarray(x[NS * k:NS * k + NS])}
        m.update(w)
        in_maps.append(m)
    res = run_bass_kernel_spmd(nc, in_maps, core_ids=list(range(NCORES)))
    return np.concatenate([r["out"] for r in res.results], axis=0)
